# revision 1
# baseline (speedup 1.0000x reference)
"""DSAttention Trainium2 kernel (8 NeuronCores, SPMD).

Sharding: batch (B=2) x head-groups (4 heads each) -> 8 cores.
Core c handles batch b=c//4, heads 4*(c%4) .. 4*(c%4)+3.

Per-core math (feature-major "transposed" layouts so softmax bias/scale land
on partition axes):
  q_t = Wq_p @ hs_b.T          [256, 2048]   (+bq per-partition)
  k_t = Wk_p @ hs_b.T          [256, 2048]   (+bk per-partition)
  v   = hs_b @ Wv_p.T          [2048, 256]   (per k-tile, with a ones column
                                              per head -> softmax denominator)
  s_t[k, q] = k_t.T q_t        per head, one k-tile x all 2048 q at a time
  e_t = exp(s_t * tau/8 + delta_k/8)         (fused ACT exp, N=1024 halves;
                                              no max-subtraction: |logits|<~12)
  ctx_t[65, q] = [v | 1].T @ e_t             accumulated over 16 k-tiles;
                                              row 64 = denominator
  ctx_t[0:64] *= 1/ctx_t[64]                 (PE rank-1 broadcast of d, then
                                              64-lane DVE reciprocal + mul)
  out_partial = ctx.T @ Wo_p.T               [2048, 1024]
Host: out[b] = sum of the 4 head-group partials + bv @ Wo.T + bo
(softmax rows sum to 1, so the v/out biases commute to the host exactly).

All matmuls in float32r (~1.2e-4 input rounding, full PE rate at N>=256).
Phase B is software-pipelined: ctx matmuls for k-tile kt-1 are emitted after
the scores matmuls for kt so the PE queue never drains waiting on ACT.
"""

import sys

for _p in ("/opt/trn_rl_repo", "/opt/pypackages"):
    if _p not in sys.path:
        sys.path.append(_p)

import numpy as np

import concourse.bass as bass
import concourse.tile as tile
from concourse import bacc, mybir
from concourse.bass_utils import run_bass_kernel_spmd

B, L, H = 2, 2048, 1024
NH, HD = 16, 64
NCORES = 8
HPC = 4  # heads per core
FPC = HPC * HD  # 256
NKT = L // 128  # 16 k-tiles
NHC = H // 128  # 8 H-contraction chunks

F32 = mybir.dt.float32
F32R = mybir.dt.float32r

_NC_CACHE = {}

# Dedup consecutive identical LDWEIGHTS in walrus codegen: every fp32r matmul
# self-loads its stationary operand, and consecutive matmuls often share it.
import concourse.bass_utils as _bu

_orig_run_command = _bu.run_command


def _run_command_ldwopt(cmd, *a, **kw):
    if isinstance(cmd, list):
        cmd = [
            "--enable-ldw-opt=true" if c == "--enable-ldw-opt=false" else c
            for c in cmd
        ]
    return _orig_run_command(cmd, *a, **kw)


_bu.run_command = _run_command_ldwopt


def _build_kernel():
    nc = bacc.Bacc(None, target_bir_lowering=False, debug=False)

    hs_t = nc.declare_dram_parameter("hs_t", [H, L], F32, isOutput=False)
    wq_t = nc.declare_dram_parameter("wq_t", [H, FPC], F32, isOutput=False)
    wk_t = nc.declare_dram_parameter("wk_t", [H, FPC], F32, isOutput=False)
    wv_t = nc.declare_dram_parameter("wv_t", [H, FPC], F32, isOutput=False)
    wo_t = nc.declare_dram_parameter("wo_t", [FPC, H], F32, isOutput=False)
    bq2 = nc.declare_dram_parameter("bq2", [128, 2], F32, isOutput=False)
    bk2 = nc.declare_dram_parameter("bk2", [128, 2], F32, isOutput=False)
    tau8 = nc.declare_dram_parameter("tau8", [128, 1], F32, isOutput=False)
    delta8 = nc.declare_dram_parameter("delta8", [128, NKT], F32, isOutput=False)
    out = nc.declare_dram_parameter("out", [L, H], F32, isOutput=True)
    scratch = nc.declare_dram_parameter("scratch", [128, 512], F32, isOutput=True)

    with tile.TileContext(nc) as tc:
        with (
            tc.tile_pool(name="persist", bufs=1) as persist,
            tc.tile_pool(name="hsw", bufs=1) as hsw,
            # PSUM: "sc" 2 x [128,1024] slots (4 banks) + "ctx" 4 x 2KB (4 banks)
            tc.tile_pool(name="sc_ps", bufs=2, space="PSUM") as sc_ps,
            tc.tile_pool(name="ctx_ps", bufs=4, space="PSUM") as ctx_ps,
            tc.tile_pool(name="work", bufs=4) as work,
            tc.tile_pool(name="dscratch", bufs=2, space="DRAM") as dscratch,
        ):
            # ---- input loads -------------------------------------------------
            hs_sb = []
            for c in range(NHC):
                t = hsw.tile([128, L], F32R, tag=f"hs{c}", name=f"hs{c}")
                nc.sync.dma_start(out=t[:], in_=hs_t[c * 128 : (c + 1) * 128, :].bitcast(F32R))
                hs_sb.append(t)
            w_sb = {}
            for name, w in (("q", wq_t), ("k", wk_t), ("v", wv_t)):
                tiles = []
                for c in range(NHC):
                    t = hsw.tile([128, FPC], F32R, tag=f"w{name}{c}", name=f"w{name}{c}")
                    nc.scalar.dma_start(out=t[:], in_=w[c * 128 : (c + 1) * 128, :].bitcast(F32R))
                    tiles.append(t)
                w_sb[name] = tiles
            wo_sb = []
            for c in range(2):
                t = persist.tile([128, H], F32R, tag=f"wo{c}", name=f"wo{c}")
                nc.scalar.dma_start(out=t[:], in_=wo_t[c * 128 : (c + 1) * 128, :].bitcast(F32R))
                wo_sb.append(t)
            bq_sb = persist.tile([128, 2], F32, tag="bq")
            nc.sync.dma_start(out=bq_sb[:], in_=bq2[:])
            bk_sb = persist.tile([128, 2], F32, tag="bk")
            nc.sync.dma_start(out=bk_sb[:], in_=bk2[:])
            tau_sb = persist.tile([128, 1], F32, tag="tau")
            nc.sync.dma_start(out=tau_sb[:], in_=tau8[:])
            del8_sb = persist.tile([128, NKT], F32, tag="del8")
            nc.sync.dma_start(out=del8_sb[:], in_=delta8[:])
            vones_f = persist.tile([128, HPC], F32, tag="vones_f")
            nc.vector.memset(vones_f[:], 1.0)

            # ---- phase A: projections ---------------------------------------
            q_sb = [persist.tile([128, L], F32R, tag=f"q{hp}", name=f"q{hp}") for hp in range(2)]
            k_sb = [persist.tile([128, L], F32R, tag=f"k{hp}", name=f"k{hp}") for hp in range(2)]
            for dst, wname, bias in ((q_sb, "q", bq_sb), (k_sb, "k", bk_sb)):
                for hp in range(2):
                    ps2 = [
                        sc_ps.tile([128, 1024], F32, tag="sc", name=f"ps_proj{half}")
                        for half in range(2)
                    ]
                    for c in range(NHC):
                        # one stationary load serves all 4 N=512 matmuls
                        for half in range(2):
                            for s2 in range(2):
                                nc.tensor.matmul(
                                    ps2[half][:, s2 * 512 : (s2 + 1) * 512],
                                    w_sb[wname][c][:, hp * 128 : (hp + 1) * 128],
                                    hs_sb[c][:, half * 1024 + s2 * 512 : half * 1024 + (s2 + 1) * 512],
                                    start=(c == 0),
                                    stop=(c == NHC - 1),
                                )
                    for half in range(2):
                        nc.vector.tensor_scalar_add(
                            dst[hp][:, half * 1024 : half * 1024 + 1024],
                            ps2[half][:],
                            bias[:, hp : hp + 1],
                        )

            # v: per k-tile [128, 4*65]; head h cols h*65..h*65+63, col h*65+64 = 1
            v_sb = [persist.tile([128, HPC * 65], F32R, tag=f"v{kt}", name=f"v{kt}") for kt in range(NKT)]
            for kt in range(NKT):
                ps = ctx_ps.tile([128, FPC], F32, tag="ctx", name="ps_vproj")
                for c in range(NHC):
                    nc.tensor.matmul(
                        ps[:],
                        hs_sb[c][:, kt * 128 : (kt + 1) * 128],
                        w_sb["v"][c][:],
                        start=(c == 0),
                        stop=(c == NHC - 1),
                    )
                v_view = v_sb[kt][:].rearrange("p (h w) -> p h w", h=HPC)
                nc.vector.tensor_copy(
                    v_view[:, :, 0:HD],
                    ps[:].rearrange("p (h w) -> p h w", h=HPC),
                )
                nc.vector.tensor_copy(v_view[:, :, HD : HD + 1].squeeze(), vones_f[:])

            # ---- phase B (+ phase C interleaved) ----------------------------
            # Structure: head x q-half x k-tile. Per k-tile: 2 scores matmuls
            # (N=512), 1 fused exp (N=1024), 2 ctx matmuls, and 1 K=128
            # "keepalive" filler matmul. The filler keeps the PE's HAM activity
            # monitor warm: K=64 scores matmuls alone do not register as busy,
            # so one throttle event would otherwise pin the phase at 1.2 GHz.
            # PSUM "ctx" tag rotation (4 slots): 2 ctx accumulators + 1 filler
            # + 1 spare used by the interleaved output-projection chunks.
            ctx_sb = [persist.tile([128, L], F32R, tag=f"ctx{hp}", name=f"ctx{hp}") for hp in range(2)]
            last_fill = [None]

            def emit_c_chunk(lts, paired=False):
                for lt in lts:
                    if paired:
                        # 2 psum tiles, one LDW per c-chunk serving both nch
                        pso = [
                            ctx_ps.tile([128, 512], F32, tag="ctx", name=f"ps_o{n}")
                            for n in range(2)
                        ]
                        for c in range(2):
                            for nch in range(2):
                                nc.tensor.matmul(
                                    pso[nch][:],
                                    ctx_sb[c][:, lt * 128 : (lt + 1) * 128],
                                    wo_sb[c][:, nch * 512 : (nch + 1) * 512],
                                    start=(c == 0),
                                    stop=(c == 1),
                                )
                        for nch in range(2):
                            o_sb = work.tile([128, 512], F32, tag="ostage", name="o_sb", bufs=3)
                            nc.vector.tensor_copy(o_sb[:], pso[nch][:])
                            nc.sync.dma_start(
                                out=out[lt * 128 : (lt + 1) * 128, nch * 512 : (nch + 1) * 512],
                                in_=o_sb[:],
                            )
                        continue
                    # serial PSUM use: one pso tile in flight at a time
                    for nch in range(2):
                        pso = ctx_ps.tile([128, 512], F32, tag="ctx", name="ps_o")
                        for c in range(2):
                            nc.tensor.matmul(
                                pso[:],
                                ctx_sb[c][:, lt * 128 : (lt + 1) * 128],
                                wo_sb[c][:, nch * 512 : (nch + 1) * 512],
                                start=(c == 0),
                                stop=(c == 1),
                            )
                        o_sb = work.tile([128, 512], F32, tag="ostage", name="o_sb", bufs=3)
                        nc.vector.tensor_copy(o_sb[:], pso[:])
                        nc.sync.dma_start(
                            out=out[lt * 128 : (lt + 1) * 128, nch * 512 : (nch + 1) * 512],
                            in_=o_sb[:],
                        )

            for h in range(HPC):
                hp, hr = divmod(h, 2)
                q_head = q_sb[hp][hr * HD : (hr + 1) * HD, :]
                k_head = k_sb[hp][hr * HD : (hr + 1) * HD, :]
                for half in range(2):
                    qoff = half * 1024
                    ctx2 = [
                        ctx_ps.tile(
                            [65, 512], F32, tag="ctx", name=f"ctx_h{h}f{half}{g2}"
                        )
                        for g2 in range(2)
                    ]
                    fill_ps = ctx_ps.tile([65, 512], F32, tag="ctx", name="fill_ps")
                    last_fill[0] = fill_ps
                    nfill = [0]

                    def emit_filler(kt0, h=h, fill_ps=fill_ps, nfill=nfill):
                        # K=128 keepalive reusing the ctx pair's stationary
                        nc.tensor.matmul(
                            fill_ps[:],
                            v_sb[kt0][:, h * 65 : (h + 1) * 65],
                            hs_sb[0][:, 0:512].bitcast(F32R),
                            start=(nfill[0] == 0),
                            stop=(nfill[0] == NKT - 2),
                            skip_group_check=True,
                        )
                        nfill[0] += 1

                    prev = None  # (kt, e)

                    def emit_ctx(prev, h=h, ctx2=ctx2):
                        kt0, e = prev
                        for g2 in range(2):
                            nc.tensor.matmul(
                                ctx2[g2][:],
                                v_sb[kt0][:, h * 65 : (h + 1) * 65],
                                e[:, g2 * 512 : (g2 + 1) * 512],
                                start=(kt0 == 0),
                                stop=(kt0 == NKT - 1),
                            )

                    for kt in range(NKT):
                        psS = sc_ps.tile([128, 1024], F32, tag="sc", name="ps_s")
                        for s2 in range(2):
                            nc.tensor.matmul(
                                psS[:, s2 * 512 : (s2 + 1) * 512],
                                k_head[:, kt * 128 : (kt + 1) * 128],
                                q_head[:, qoff + s2 * 512 : qoff + (s2 + 1) * 512],
                                start=True,
                                stop=True,
                            )
                        if h == HPC - 1 and half == 1 and kt == 6:
                            emit_c_chunk(range(0, 8))
                        if prev is not None:
                            emit_ctx(prev)
                            emit_filler(prev[0])
                        e_t = work.tile([128, 1024], F32R, tag="e", name="e_t", bufs=3)
                        nc.scalar.activation(
                            e_t[:],
                            psS[:],
                            mybir.ActivationFunctionType.Exp,
                            bias=del8_sb[:, kt : kt + 1],
                            scale=tau_sb[:],
                        )
                        prev = (kt, e_t)
                    emit_ctx(prev)

                    # normalize ctx[0:64] / ctx[64]: drain PSUM -> SBUF at once
                    # (frees accumulator banks), then broadcast the denominator
                    # row via DRAM-bounce DMA and divide on DVE — no PE/PSUM.
                    raws = []
                    for g2 in range(2):
                        raw = work.tile([65, 512], F32R, tag="raw", name=f"raw{g2}", bufs=2)
                        nc.vector.tensor_copy(raw[:], ctx2[g2][:])
                        raws.append(raw)
                    for g2 in range(2):
                        g_abs = half * 2 + g2
                        d_dram = dscratch.tile([1, 512], F32, tag="ddram", name="d_dram")
                        nc.sync.dma_start(out=d_dram[:], in_=raws[g2][64:65, :].bitcast(F32))
                        d_bc = work.tile([64, 512], F32, tag="dbc", name="d_bc", bufs=2)
                        nc.sync.dma_start(
                            out=d_bc[:],
                            in_=d_dram[0:1, :].to_broadcast([64, 512]),
                        )
                        r_sb = work.tile([64, 512], F32, tag="r", name="r_sb", bufs=2)
                        nc.vector.reciprocal(r_sb[:], d_bc[:])
                        nc.vector.tensor_mul(
                            ctx_sb[hp][hr * HD : (hr + 1) * HD, g_abs * 512 : (g_abs + 1) * 512],
                            raws[g2][0:64, :],
                            r_sb[:],
                        )
            emit_c_chunk(range(8, 16), paired=True)

            # read the last filler accumulator so DCE keeps the keepalives
            fcopy = work.tile([65, 512], F32, tag="ostage", name="fcopy", bufs=3)
            nc.vector.tensor_copy(fcopy[:], last_fill[0][:])
            nc.sync.dma_start(out=scratch[0:65, :], in_=fcopy[:])

    nc.compile()
    return nc


def _get_nc():
    if "nc" not in _NC_CACHE:
        _NC_CACHE["nc"] = _build_kernel()
    return _NC_CACHE["nc"]


def _make_in_maps(hidden_states, tau, delta, Wq, Wk, Wv, Wo, bq, bk):
    in_maps = []
    for c in range(NCORES):
        b, hg = divmod(c, HPC)
        fs = slice(hg * FPC, (hg + 1) * FPC)
        in_maps.append(
            {
                "hs_t": np.ascontiguousarray(hidden_states[b].T),
                "wq_t": np.ascontiguousarray(Wq[fs, :].T),
                "wk_t": np.ascontiguousarray(Wk[fs, :].T),
                "wv_t": np.ascontiguousarray(Wv[fs, :].T),
                "wo_t": np.ascontiguousarray(Wo[:, fs].T),
                "bq2": np.ascontiguousarray(bq[fs].reshape(2, 128).T),
                "bk2": np.ascontiguousarray(bk[fs].reshape(2, 128).T),
                "tau8": np.full((128, 1), tau[b, 0] / 8.0, dtype=np.float32),
                "delta8": np.ascontiguousarray((delta[b] / 8.0).reshape(NKT, 128).T),
            }
        )
    return in_maps


def kernel(hidden_states, tau, delta, Wq, bq, Wk, bk, Wv, bv, Wo, bo, _trace=False):
    hidden_states = np.asarray(hidden_states, dtype=np.float32)
    tau = np.asarray(tau, dtype=np.float32)
    delta = np.asarray(delta, dtype=np.float32)
    Wq = np.asarray(Wq, dtype=np.float32)
    Wk = np.asarray(Wk, dtype=np.float32)
    Wv = np.asarray(Wv, dtype=np.float32)
    Wo = np.asarray(Wo, dtype=np.float32)
    bq = np.asarray(bq, dtype=np.float32)
    bk = np.asarray(bk, dtype=np.float32)
    bv = np.asarray(bv, dtype=np.float32)
    bo = np.asarray(bo, dtype=np.float32)

    nc = _get_nc()
    in_maps = _make_in_maps(hidden_states, tau, delta, Wq, Wk, Wv, Wo, bq, bk)
    res = run_bass_kernel_spmd(nc, in_maps, list(range(NCORES)), trace=_trace)

    out = np.zeros((B, L, H), dtype=np.float32)
    for c in range(NCORES):
        out[c // HPC] += res.results[c]["out"]
    # v/out-proj biases commute through softmax-normalized attention exactly
    out += bv @ Wo.T + bo
    if _trace:
        kernel._last_exec_time_ns = res.exec_time_ns
        kernel._last_profile_json = res.profile_json
    return out

